# revision 1
# baseline (speedup 1.0000x reference)
# Task-aware MoE layer (top-2 of 8 experts, 1024->4096->1024 MLPs) on 8
# Trainium2 NeuronCores.
#
# Sharding: expert-parallel. Core e holds expert e's weights; the host
# computes the gating (bitwise-identical to the reference ops) and plays
# the role of the all-to-all token dispatch/return by gathering each
# expert's tokens into a padded batch for its core and scatter-adding the
# weighted outputs back. The device does 99.97% of the FLOPs (the two big
# matmuls per expert) in bf16 with fp32 accumulation.
#
# Fast path ("folded"): when b1 == b2 == 0 (always true for this module's
# init), the per-token top-k softmax weight cv > 0 commutes with relu:
#   cv * (relu(x W1^T) W2^T) = relu((cv*x) W1^T) W2^T
# so cv is folded into the gathered tokens on the host and the device does
# two plain matmuls + relu per expert. A general path (biases via ACT
# bias / a K=1 ones-matmul, cv applied via per-partition ACT scale) covers
# nonzero biases.

import numpy as np
import ml_dtypes

import concourse.tile as tile
from concourse import bacc, mybir
from concourse.bass_utils import run_bass_kernel_spmd

NUM_EXPERTS = 8
NUM_TASKS = 8
TOP_K = 2
D_IN = 1024
D_HID = 4096
D_OUT = 1024

AF = mybir.ActivationFunctionType
BF16 = mybir.dt.bfloat16
F32 = mybir.dt.float32

# Populated by kernel() with the BassKernelResults of the device run, so a
# test harness can read exec_time_ns / profile paths.
LAST_RESULTS = None

_KERNEL_CACHE = {}


def _emit_body(nc, tc, pools, C, tensors, folded, cb=512):
    """One full forward pass over the C-token batch.

    Token blocks of cb, each split into <=512-column chunks. Within a
    block, the k-loops keep the stationary operand fixed across the chunks
    (layer 1, cb>512 only) / the two O-halves (layer 2) so consecutive
    matmuls share their LDWEIGHTS; _dedupe_ldweights() then drops the
    redundant reloads. For cb<=512 w1 is SBUF-resident; for cb=1024 the
    hT tile grows, so w1 streams through a rolling window instead.
    """
    wpool, w1pool, xpool, hpool, pspool, ps2pool, ypool = pools
    MT = C // 128
    CB = cb
    stream_w1 = CB > 512
    blocks = [(i * CB, CB) for i in range(C // CB)]
    if C % CB:
        blocks.append((C - C % CB, C % CB))

    def chunks_of(cw):
        out = [(i * 512, 512) for i in range(cw // 512)]
        if cw % 512:
            out.append((cw - cw % 512, cw % 512))
        return out

    # First token block first so PE can start as soon as w1's first
    # m-chunk lands; w1 arrives in per-m chunks consumed in order.
    xt0 = xpool.tile([128, 8, CB], BF16, tag="xt")
    nc.sync.dma_start(xt0[:, :, : blocks[0][1]], tensors["xa"][:, :, : blocks[0][1]])

    if not stream_w1:
        w1t = wpool.tile([128, 32, 8, 128], BF16, tag="w1")
        for m in range(32):
            nc.sync.dma_start(w1t[:, m], tensors["w1a"][m])
    # w2 per-k chunks: layer 2 consumes them k-ascending well after layer 1
    # has started. When w1 streams, defer the w2 loads until after block
    # 0's layer-1 DMAs are queued so they don't stall PE startup.
    w2t = wpool.tile([128, 32, D_OUT], BF16, tag="w2")
    if not stream_w1:
        for k in range(32):
            nc.sync.dma_start(w2t[:, k], tensors["w2a"][k])

    if not folded:
        b1t = wpool.tile([128, 32], F32, tag="b1")
        nc.sync.dma_start(b1t[:], tensors["b1a"][:])
        b2t = wpool.tile([1, D_OUT], BF16, tag="b2")
        nc.sync.dma_start(b2t[:], tensors["b2a"][:])
        cvt = wpool.tile([128, MT], F32, tag="cv")
        nc.sync.dma_start(cvt[:], tensors["cva"][:])
        ones1 = wpool.tile([1, 128], BF16, tag="ones")
        nc.vector.memset(ones1[:], 1.0)

    yo = tensors["y"]
    for bi, (c0, cw) in enumerate(blocks):
        if bi == 0:
            xt = xt0
        else:
            xt = xpool.tile([128, 8, CB], BF16, tag="xt")
            nc.sync.dma_start(xt[:, :, :cw], tensors["xa"][:, :, c0 : c0 + cw])

        # Layer 1: hT[m*128+p, c] = relu(sum_d W1[m*128+p, d] x[c, d] (+ b1))
        hT = hpool.tile([128, 32, CB], BF16, tag="hT")
        chunks = chunks_of(cw)
        for m in range(32):
            if stream_w1:
                w1c = w1pool.tile([128, 8, 128], BF16, tag="w1s")
                nc.sync.dma_start(w1c[:], tensors["w1a"][m])
            else:
                w1c = w1t[:, m]
            pss = []
            for ci in range(len(chunks)):
                pss.append(pspool.tile([128, 512], F32, tag="ps", name=f"ps{ci}"))
            for k in range(8):
                for ci, (o, w) in enumerate(chunks):
                    nc.tensor.matmul(
                        pss[ci][:, :w],
                        w1c[:, k],
                        xt[:, k, o : o + w],
                        start=(k == 0),
                        stop=(k == 7),
                    )
            for ci, (o, w) in enumerate(chunks):
                if folded:
                    nc.scalar.activation(hT[:, m, o : o + w], pss[ci][:, :w], AF.Relu)
                else:
                    nc.scalar.activation(
                        hT[:, m, o : o + w],
                        pss[ci][:, :w],
                        AF.Relu,
                        bias=b1t[:, m : m + 1],
                    )

        if stream_w1 and bi == 0:
            for k in range(32):
                nc.sync.dma_start(w2t[:, k], tensors["w2a"][k])

        # Layer 2: y[c, o] = (cv[c] *) (sum_h hT[h, c] W2[o, h] (+ b2[o]))
        for mt in range(cw // 128):
            gmt = c0 // 128 + mt
            yt = ypool.tile([128, D_OUT], F32, tag="yt")
            ps2a = ps2pool.tile([128, 512], F32, tag="ps2", name="ps2a")
            ps2b = ps2pool.tile([128, 512], F32, tag="ps2", name="ps2b")
            for k in range(32):
                nc.tensor.matmul(
                    ps2a[:],
                    hT[:, k, mt * 128 : (mt + 1) * 128],
                    w2t[:, k, 0:512],
                    start=(k == 0),
                    stop=(folded and k == 31),
                )
                nc.tensor.matmul(
                    ps2b[:],
                    hT[:, k, mt * 128 : (mt + 1) * 128],
                    w2t[:, k, 512:1024],
                    start=(k == 0),
                    stop=(folded and k == 31),
                )
            for n, ps2 in enumerate((ps2a, ps2b)):
                if folded:
                    nc.vector.tensor_copy(yt[:, n * 512 : (n + 1) * 512], ps2[:])
                else:
                    nc.tensor.matmul(
                        ps2[:],
                        ones1[:],
                        b2t[:, n * 512 : (n + 1) * 512],
                        start=False,
                        stop=True,
                    )
                    nc.scalar.activation(
                        yt[:, n * 512 : (n + 1) * 512],
                        ps2[:],
                        AF.Copy,
                        scale=cvt[:, gmt : gmt + 1],
                    )
            nc.sync.dma_start(yo[gmt], yt[:])


_PASSTHROUGH_TYPES = {
    "InstMatmult",
    "InstDMACopy",
    "InstActivation",
    "InstTensorCopy",
    "InstMemset",
    "InstEventSemaphore",
    "InstRegisterMove",
}


def _dedupe_ldweights(nc):
    """Drop InstLdweights that reload the exact weights AP loaded by the
    immediately preceding InstLdweights (with only PE matmuls / non-PE work
    in between). Safe post-scheduling: Tile assigns LDWs no semaphore
    updates (only matmuls inc the PE sem), so removal does not perturb sem
    numbering; LDWs carrying waits or updates are kept."""
    removed = 0
    for f in nc.m.functions:
        for b in f.blocks:
            insts = b.instructions
            keep = []
            last_sig = None
            for i in insts:
                t = type(i).__name__
                if t == "InstLdweights":
                    ap = i.ins[0]
                    sig = (ap.memref, ap.offset, str(ap.ap))
                    si = i.sync_info
                    clean = not si or (not si.on_wait and not si.on_update)
                    if clean and sig == last_sig:
                        removed += 1
                        continue
                    last_sig = sig
                elif t not in _PASSTHROUGH_TYPES:
                    last_sig = None
                keep.append(i)
            if len(keep) != len(insts):
                b.instructions = keep
    return removed


def _build_device_kernel(C, reps=1, folded=True, unroll=1, cb=512):
    """Per-core SPMD kernel over a padded batch of C tokens, activations
    kept feature-major between the layers so every matmul contracts on the
    partition axis.

    DRAM inputs (per core, host-prepped layouts):
      w1a [32, 128, 8, 128] bf16 : w1a[m,p,k,q] = W1[m*128+q, k*128+p]  (W1 [4096,1024])
      w2a [32, 128, 1024]   bf16 : w2a[k,p,o]   = W2[o, k*128+p]        (W2 [1024,4096])
      xa  [128, 8, C]       bf16 : xa[p,k,c]    = xtok[c, k*128+p]      (cv pre-folded)
      (general path only) b1a [128, 32] f32, b2a [1, 1024] bf16, cva [128, MT] f32
    DRAM output:
      y   [MT, 128, 1024]   f32  : y[mt, p, :]  = out token mt*128+p

    reps > 1 wraps the whole body in an on-device loop (for differential
    wall-clock timing of the NEFF without NTFF profiling).
    """
    assert C % 128 == 0
    MT = C // 128

    nc = bacc.Bacc(
        "TRN2", target_bir_lowering=False, debug=False, num_devices=NUM_EXPERTS
    )
    tensors = {
        "w1a": nc.dram_tensor("w1a", [32, 128, 8, 128], BF16, kind="ExternalInput"),
        "w2a": nc.dram_tensor("w2a", [32, 128, D_OUT], BF16, kind="ExternalInput"),
        "xa": nc.dram_tensor("xa", [128, 8, C], BF16, kind="ExternalInput"),
        "y": nc.dram_tensor("y", [MT, 128, D_OUT], F32, kind="ExternalOutput"),
    }
    if not folded:
        tensors["b1a"] = nc.dram_tensor("b1a", [128, 32], F32, kind="ExternalInput")
        tensors["b2a"] = nc.dram_tensor("b2a", [1, D_OUT], BF16, kind="ExternalInput")
        tensors["cva"] = nc.dram_tensor("cva", [128, MT], F32, kind="ExternalInput")

    ps_bufs = 4 if cb <= 512 else 6
    ps2_bufs = 4 if cb <= 512 else 2
    with tile.TileContext(nc) as tc:
        with (
            tc.tile_pool(name="w", bufs=1) as wpool,
            tc.tile_pool(name="w1s", bufs=10) as w1pool,
            tc.tile_pool(name="xin", bufs=2) as xpool,
            tc.tile_pool(name="h", bufs=1) as hpool,
            tc.tile_pool(name="ps", bufs=ps_bufs, space="PSUM") as pspool,
            tc.tile_pool(name="ps2", bufs=ps2_bufs, space="PSUM") as ps2pool,
            tc.tile_pool(name="yout", bufs=2) as ypool,
        ):
            pools = (wpool, w1pool, xpool, hpool, pspool, ps2pool, ypool)
            if reps == 1:
                for _ in range(unroll):
                    _emit_body(nc, tc, pools, C, tensors, folded, cb)
            else:
                with tc.For_i(0, reps, 1):
                    for _ in range(unroll):
                        _emit_body(nc, tc, pools, C, tensors, folded, cb)
    _dedupe_ldweights(nc)
    nc.finalize()
    return nc


def _gating_combine(x, gating_w):
    """Dense [N, E] combine matrix, replicating the reference ops exactly
    (same jax backend as the harness's reference run -> identical top-k)."""
    import jax
    import jax.numpy as jnp

    logits = jnp.einsum("btd,ted->bte", x, gating_w)
    topk_vals, topk_idx = jax.lax.top_k(logits, TOP_K)
    topk_w = jax.nn.softmax(topk_vals, axis=-1)

    B, T, _ = x.shape
    N = B * T
    idx_flat = np.asarray(topk_idx).reshape(N, TOP_K)
    w_flat = np.asarray(topk_w, dtype=np.float32).reshape(N, TOP_K)
    combine = np.zeros((N, NUM_EXPERTS), dtype=np.float32)
    rows = np.arange(N)[:, None]
    np.add.at(combine, (rows, idx_flat), w_flat)
    return combine


def _make_in_maps(x_flat, combine, w1, b1, w2, b2, ids, counts, C, folded):
    MT = C // 128
    in_maps = []
    for e in range(NUM_EXPERTS):
        cnt = counts[e]
        cv = combine[ids[e], e].astype(np.float32)
        xg32 = x_flat[ids[e]]
        if folded:
            xg32 = xg32 * cv[:, None]
        xg = np.zeros((C, D_IN), dtype=ml_dtypes.bfloat16)
        xg[:cnt] = xg32

        # xa[p, k, c] = xg[c, k*128+p]
        xa = np.ascontiguousarray(xg.reshape(C, 8, 128).transpose(2, 1, 0))
        # w1a[m, p, k, q] = w1[e][m*128+q, k*128+p]
        w1a = np.ascontiguousarray(
            w1[e]
            .astype(ml_dtypes.bfloat16)
            .reshape(32, 128, 8, 128)
            .transpose(0, 3, 2, 1)
        )
        # w2a[k, p, o] = w2[e][o, k*128+p]
        w2a = np.ascontiguousarray(
            w2[e].astype(ml_dtypes.bfloat16).reshape(D_OUT, 32, 128).transpose(1, 2, 0)
        )
        m = {"w1a": w1a, "w2a": w2a, "xa": xa}
        if not folded:
            cvp = np.zeros(C, dtype=np.float32)
            cvp[:cnt] = cv
            m["b1a"] = np.ascontiguousarray(b1[e].reshape(32, 128).T.astype(np.float32))
            m["b2a"] = b2[e].astype(ml_dtypes.bfloat16).reshape(1, D_OUT)
            m["cva"] = np.ascontiguousarray(cvp.reshape(MT, 128).T)
        in_maps.append(m)
    return in_maps


def kernel(x, gating_w, w1, b1, w2, b2, trace=False, reps=1):
    global LAST_RESULTS
    x = np.asarray(x, dtype=np.float32)
    gating_w = np.asarray(gating_w, dtype=np.float32)
    w1 = np.asarray(w1, dtype=np.float32)
    b1 = np.asarray(b1, dtype=np.float32)
    w2 = np.asarray(w2, dtype=np.float32)
    b2 = np.asarray(b2, dtype=np.float32)

    B, T, D = x.shape
    N = B * T
    x_flat = x.reshape(N, D)

    combine = _gating_combine(x, gating_w)

    # Token dispatch (host-side all-to-all): gather each expert's tokens.
    ids = [np.nonzero(combine[:, e])[0] for e in range(NUM_EXPERTS)]
    counts = [len(i) for i in ids]
    C = max(128, -(-max(counts) // 128) * 128)

    folded = not (b1.any() or b2.any())
    in_maps = _make_in_maps(
        x_flat, combine, w1, b1, w2, b2, ids, counts, C, folded
    )

    key = (C, reps, folded)
    if key not in _KERNEL_CACHE:
        _KERNEL_CACHE[key] = _build_device_kernel(C, reps, folded)
    nc = _KERNEL_CACHE[key]

    res = run_bass_kernel_spmd(nc, in_maps, list(range(NUM_EXPERTS)), trace=trace)
    LAST_RESULTS = res

    # Token return (host-side all-to-all back) + combine. Within one expert
    # the token ids are unique, so fancy-index += is safe.
    out = np.zeros((N, D_OUT), dtype=np.float32)
    for e in range(NUM_EXPERTS):
        y = np.asarray(res.results[e]["y"], dtype=np.float32).reshape(C, D_OUT)
        out[ids[e]] += y[: counts[e]]
    return out.reshape(B, T, D_OUT)



# revision 2
# speedup vs baseline: 10.8511x; 10.8511x over previous
# Task-aware MoE layer (top-2 of 8 experts, 1024->4096->1024 MLPs) on 8
# Trainium2 NeuronCores.
#
# Sharding: expert-parallel. Core e holds expert e's weights; the host
# computes the gating (bitwise-identical to the reference ops) and plays
# the role of the all-to-all token dispatch/return by gathering each
# expert's tokens into a padded batch for its core and scatter-adding the
# weighted outputs back. The device does 99.97% of the FLOPs (the two big
# matmuls per expert) in bf16 with fp32 accumulation.
#
# Fast path ("folded"): when b1 == b2 == 0 (always true for this module's
# init), the per-token top-k softmax weight cv > 0 commutes with relu:
#   cv * (relu(x W1^T) W2^T) = relu((cv*x) W1^T) W2^T
# so cv is folded into the gathered tokens on the host and the device does
# two plain matmuls + relu per expert. A general path (biases via ACT
# bias / a K=1 ones-matmul, cv applied via per-partition ACT scale) covers
# nonzero biases.

import numpy as np
import ml_dtypes

import concourse.tile as tile
from concourse import bacc, mybir
from concourse.bass_utils import run_bass_kernel_spmd

NUM_EXPERTS = 8
NUM_TASKS = 8
TOP_K = 2
D_IN = 1024
D_HID = 4096
D_OUT = 1024

AF = mybir.ActivationFunctionType
BF16 = mybir.dt.bfloat16
F32 = mybir.dt.float32

# Populated by kernel() with the BassKernelResults of the device run, so a
# test harness can read exec_time_ns / profile paths.
LAST_RESULTS = None

_KERNEL_CACHE = {}


def _emit_body(nc, tc, pools, C, tensors, folded, cb=512):
    """One full forward pass over the C-token batch.

    Token blocks of cb, each split into <=512-column chunks. Within a
    block, the k-loops keep the stationary operand fixed across the chunks
    (layer 1, cb>512 only) / the two O-halves (layer 2) so consecutive
    matmuls share their LDWEIGHTS; _dedupe_ldweights() then drops the
    redundant reloads. For cb<=512 w1 is SBUF-resident; for cb=1024 the
    hT tile grows, so w1 streams through a rolling window instead.
    """
    wpool, w1pool, xpool, hpool, pspool, ps2pool, ypool = pools
    MT = C // 128
    CB = cb
    stream_w1 = CB > 512
    blocks = [(i * CB, CB) for i in range(C // CB)]
    if C % CB:
        blocks.append((C - C % CB, C % CB))

    def chunks_of(cw):
        out = [(i * 512, 512) for i in range(cw // 512)]
        if cw % 512:
            out.append((cw - cw % 512, cw % 512))
        return out

    # First token block first so PE can start as soon as w1's first
    # m-chunk lands; w1 arrives in per-m chunks consumed in order.
    xt0 = xpool.tile([128, 8, CB], BF16, tag="xt")
    nc.sync.dma_start(xt0[:, :, : blocks[0][1]], tensors["xa"][:, :, : blocks[0][1]])

    if not stream_w1:
        w1t = wpool.tile([128, 32, 8, 128], BF16, tag="w1")
        for m in range(32):
            nc.sync.dma_start(w1t[:, m], tensors["w1a"][m])
    # w2 per-k chunks: layer 2 consumes them k-ascending well after layer 1
    # has started. When w1 streams, defer the w2 loads until after block
    # 0's layer-1 DMAs are queued so they don't stall PE startup.
    w2t = wpool.tile([128, 32, D_OUT], BF16, tag="w2")
    if not stream_w1:
        for k in range(32):
            nc.sync.dma_start(w2t[:, k], tensors["w2a"][k])

    if not folded:
        b1t = wpool.tile([128, 32], F32, tag="b1")
        nc.sync.dma_start(b1t[:], tensors["b1a"][:])
        b2t = wpool.tile([1, D_OUT], BF16, tag="b2")
        nc.sync.dma_start(b2t[:], tensors["b2a"][:])
        cvt = wpool.tile([128, MT], F32, tag="cv")
        nc.sync.dma_start(cvt[:], tensors["cva"][:])
        ones1 = wpool.tile([1, 128], BF16, tag="ones")
        nc.vector.memset(ones1[:], 1.0)

    yo = tensors["y"]
    for bi, (c0, cw) in enumerate(blocks):
        if bi == 0:
            xt = xt0
        else:
            xt = xpool.tile([128, 8, CB], BF16, tag="xt")
            nc.sync.dma_start(xt[:, :, :cw], tensors["xa"][:, :, c0 : c0 + cw])

        # Layer 1: hT[m*128+p, c] = relu(sum_d W1[m*128+p, d] x[c, d] (+ b1))
        hT = hpool.tile([128, 32, CB], BF16, tag="hT")
        chunks = chunks_of(cw)
        for m in range(32):
            if stream_w1:
                w1c = w1pool.tile([128, 8, 128], BF16, tag="w1s")
                nc.sync.dma_start(w1c[:], tensors["w1a"][m])
            else:
                w1c = w1t[:, m]
            pss = []
            for ci in range(len(chunks)):
                pss.append(pspool.tile([128, 512], F32, tag="ps", name=f"ps{ci}"))
            for k in range(8):
                for ci, (o, w) in enumerate(chunks):
                    nc.tensor.matmul(
                        pss[ci][:, :w],
                        w1c[:, k],
                        xt[:, k, o : o + w],
                        start=(k == 0),
                        stop=(k == 7),
                    )
            for ci, (o, w) in enumerate(chunks):
                if folded:
                    nc.scalar.activation(hT[:, m, o : o + w], pss[ci][:, :w], AF.Relu)
                else:
                    nc.scalar.activation(
                        hT[:, m, o : o + w],
                        pss[ci][:, :w],
                        AF.Relu,
                        bias=b1t[:, m : m + 1],
                    )

        if stream_w1 and bi == 0:
            for k in range(32):
                nc.sync.dma_start(w2t[:, k], tensors["w2a"][k])

        # Layer 2: y[c, o] = (cv[c] *) (sum_h hT[h, c] W2[o, h] (+ b2[o]))
        # The k-loop is innermost per O-half so the stationary operand
        # (hT[:, k]) changes on EVERY consecutive matmul: alternating
        # LDWEIGHTS streams measure ~120 ns/512-col MM on this HW vs ~205
        # ns when one lhsT is reused for two MMs.
        for mt in range(cw // 128):
            gmt = c0 // 128 + mt
            yt = ypool.tile([128, D_OUT], F32, tag="yt")
            ps2a = ps2pool.tile([128, 512], F32, tag="ps2", name="ps2a")
            ps2b = ps2pool.tile([128, 512], F32, tag="ps2", name="ps2b")
            for n, ps2 in enumerate((ps2a, ps2b)):
                for k in range(32):
                    nc.tensor.matmul(
                        ps2[:],
                        hT[:, k, mt * 128 : (mt + 1) * 128],
                        w2t[:, k, n * 512 : (n + 1) * 512],
                        start=(k == 0),
                        stop=(folded and k == 31),
                    )
            for n, ps2 in enumerate((ps2a, ps2b)):
                if folded:
                    nc.vector.tensor_copy(yt[:, n * 512 : (n + 1) * 512], ps2[:])
                else:
                    nc.tensor.matmul(
                        ps2[:],
                        ones1[:],
                        b2t[:, n * 512 : (n + 1) * 512],
                        start=False,
                        stop=True,
                    )
                    nc.scalar.activation(
                        yt[:, n * 512 : (n + 1) * 512],
                        ps2[:],
                        AF.Copy,
                        scale=cvt[:, gmt : gmt + 1],
                    )
            nc.sync.dma_start(yo[gmt], yt[:])


_PASSTHROUGH_TYPES = {
    "InstMatmult",
    "InstDMACopy",
    "InstActivation",
    "InstTensorCopy",
    "InstMemset",
    "InstEventSemaphore",
    "InstRegisterMove",
}


def _dedupe_ldweights(nc):
    """Drop InstLdweights that reload the exact weights AP loaded by the
    immediately preceding InstLdweights (with only PE matmuls / non-PE work
    in between). Safe post-scheduling: Tile assigns LDWs no semaphore
    updates (only matmuls inc the PE sem), so removal does not perturb sem
    numbering; LDWs carrying waits or updates are kept."""
    removed = 0
    for f in nc.m.functions:
        for b in f.blocks:
            insts = b.instructions
            keep = []
            last_sig = None
            for i in insts:
                t = type(i).__name__
                if t == "InstLdweights":
                    ap = i.ins[0]
                    sig = (ap.memref, ap.offset, str(ap.ap))
                    si = i.sync_info
                    clean = not si or (not si.on_wait and not si.on_update)
                    if clean and sig == last_sig:
                        removed += 1
                        continue
                    last_sig = sig
                elif t not in _PASSTHROUGH_TYPES:
                    last_sig = None
                keep.append(i)
            if len(keep) != len(insts):
                b.instructions = keep
    return removed


def _build_device_kernel(C, reps=1, folded=True, unroll=1, cb=512):
    """Per-core SPMD kernel over a padded batch of C tokens, activations
    kept feature-major between the layers so every matmul contracts on the
    partition axis.

    DRAM inputs (per core, host-prepped layouts):
      w1a [32, 128, 8, 128] bf16 : w1a[m,p,k,q] = W1[m*128+q, k*128+p]  (W1 [4096,1024])
      w2a [32, 128, 1024]   bf16 : w2a[k,p,o]   = W2[o, k*128+p]        (W2 [1024,4096])
      xa  [128, 8, C]       bf16 : xa[p,k,c]    = xtok[c, k*128+p]      (cv pre-folded)
      (general path only) b1a [128, 32] f32, b2a [1, 1024] bf16, cva [128, MT] f32
    DRAM output:
      y   [MT, 128, 1024]   f32  : y[mt, p, :]  = out token mt*128+p

    reps > 1 wraps the whole body in an on-device loop (for differential
    wall-clock timing of the NEFF without NTFF profiling).
    """
    assert C % 128 == 0
    MT = C // 128

    nc = bacc.Bacc(
        "TRN2", target_bir_lowering=False, debug=False, num_devices=NUM_EXPERTS
    )
    tensors = {
        "w1a": nc.dram_tensor("w1a", [32, 128, 8, 128], BF16, kind="ExternalInput"),
        "w2a": nc.dram_tensor("w2a", [32, 128, D_OUT], BF16, kind="ExternalInput"),
        "xa": nc.dram_tensor("xa", [128, 8, C], BF16, kind="ExternalInput"),
        "y": nc.dram_tensor("y", [MT, 128, D_OUT], F32, kind="ExternalOutput"),
    }
    if not folded:
        tensors["b1a"] = nc.dram_tensor("b1a", [128, 32], F32, kind="ExternalInput")
        tensors["b2a"] = nc.dram_tensor("b2a", [1, D_OUT], BF16, kind="ExternalInput")
        tensors["cva"] = nc.dram_tensor("cva", [128, MT], F32, kind="ExternalInput")

    ps_bufs = 4 if cb <= 512 else 6
    ps2_bufs = 4 if cb <= 512 else 2
    with tile.TileContext(nc) as tc:
        with (
            tc.tile_pool(name="w", bufs=1) as wpool,
            tc.tile_pool(name="w1s", bufs=10) as w1pool,
            tc.tile_pool(name="xin", bufs=2) as xpool,
            tc.tile_pool(name="h", bufs=1) as hpool,
            tc.tile_pool(name="ps", bufs=ps_bufs, space="PSUM") as pspool,
            tc.tile_pool(name="ps2", bufs=ps2_bufs, space="PSUM") as ps2pool,
            tc.tile_pool(name="yout", bufs=2) as ypool,
        ):
            pools = (wpool, w1pool, xpool, hpool, pspool, ps2pool, ypool)
            if reps == 1:
                for _ in range(unroll):
                    _emit_body(nc, tc, pools, C, tensors, folded, cb)
            else:
                with tc.For_i(0, reps, 1):
                    for _ in range(unroll):
                        _emit_body(nc, tc, pools, C, tensors, folded, cb)
    _dedupe_ldweights(nc)
    nc.finalize()
    return nc


def _gating_combine(x, gating_w):
    """Dense [N, E] combine matrix, replicating the reference ops exactly
    (same jax backend as the harness's reference run -> identical top-k)."""
    import jax
    import jax.numpy as jnp

    logits = jnp.einsum("btd,ted->bte", x, gating_w)
    topk_vals, topk_idx = jax.lax.top_k(logits, TOP_K)
    topk_w = jax.nn.softmax(topk_vals, axis=-1)

    B, T, _ = x.shape
    N = B * T
    idx_flat = np.asarray(topk_idx).reshape(N, TOP_K)
    w_flat = np.asarray(topk_w, dtype=np.float32).reshape(N, TOP_K)
    combine = np.zeros((N, NUM_EXPERTS), dtype=np.float32)
    rows = np.arange(N)[:, None]
    np.add.at(combine, (rows, idx_flat), w_flat)
    return combine


def _make_in_maps(x_flat, combine, w1, b1, w2, b2, ids, counts, C, folded):
    MT = C // 128
    in_maps = []
    for e in range(NUM_EXPERTS):
        cnt = counts[e]
        cv = combine[ids[e], e].astype(np.float32)
        xg32 = x_flat[ids[e]]
        if folded:
            xg32 = xg32 * cv[:, None]
        xg = np.zeros((C, D_IN), dtype=ml_dtypes.bfloat16)
        xg[:cnt] = xg32

        # xa[p, k, c] = xg[c, k*128+p]
        xa = np.ascontiguousarray(xg.reshape(C, 8, 128).transpose(2, 1, 0))
        # w1a[m, p, k, q] = w1[e][m*128+q, k*128+p]
        w1a = np.ascontiguousarray(
            w1[e]
            .astype(ml_dtypes.bfloat16)
            .reshape(32, 128, 8, 128)
            .transpose(0, 3, 2, 1)
        )
        # w2a[k, p, o] = w2[e][o, k*128+p]
        w2a = np.ascontiguousarray(
            w2[e].astype(ml_dtypes.bfloat16).reshape(D_OUT, 32, 128).transpose(1, 2, 0)
        )
        m = {"w1a": w1a, "w2a": w2a, "xa": xa}
        if not folded:
            cvp = np.zeros(C, dtype=np.float32)
            cvp[:cnt] = cv
            m["b1a"] = np.ascontiguousarray(b1[e].reshape(32, 128).T.astype(np.float32))
            m["b2a"] = b2[e].astype(ml_dtypes.bfloat16).reshape(1, D_OUT)
            m["cva"] = np.ascontiguousarray(cvp.reshape(MT, 128).T)
        in_maps.append(m)
    return in_maps


def kernel(x, gating_w, w1, b1, w2, b2, trace=False, reps=1):
    global LAST_RESULTS
    x = np.asarray(x, dtype=np.float32)
    gating_w = np.asarray(gating_w, dtype=np.float32)
    w1 = np.asarray(w1, dtype=np.float32)
    b1 = np.asarray(b1, dtype=np.float32)
    w2 = np.asarray(w2, dtype=np.float32)
    b2 = np.asarray(b2, dtype=np.float32)

    B, T, D = x.shape
    N = B * T
    x_flat = x.reshape(N, D)

    combine = _gating_combine(x, gating_w)

    # Token dispatch (host-side all-to-all): gather each expert's tokens.
    ids = [np.nonzero(combine[:, e])[0] for e in range(NUM_EXPERTS)]
    counts = [len(i) for i in ids]
    C = max(128, -(-max(counts) // 128) * 128)

    folded = not (b1.any() or b2.any())
    in_maps = _make_in_maps(
        x_flat, combine, w1, b1, w2, b2, ids, counts, C, folded
    )

    key = (C, reps, folded)
    if key not in _KERNEL_CACHE:
        _KERNEL_CACHE[key] = _build_device_kernel(C, reps, folded)
    nc = _KERNEL_CACHE[key]

    res = run_bass_kernel_spmd(nc, in_maps, list(range(NUM_EXPERTS)), trace=trace)
    LAST_RESULTS = res

    # Token return (host-side all-to-all back) + combine. Within one expert
    # the token ids are unique, so fancy-index += is safe.
    out = np.zeros((N, D_OUT), dtype=np.float32)
    for e in range(NUM_EXPERTS):
        y = np.asarray(res.results[e]["y"], dtype=np.float32).reshape(C, D_OUT)
        out[ids[e]] += y[: counts[e]]
    return out.reshape(B, T, D_OUT)

